# revision 62
# baseline (speedup 1.0000x reference)
"""Multi-head attention (QK-LayerNorm, causal) Trainium2 kernel over 8 NeuronCores.

Sharding: tensor-parallel over heads - 2 heads per core. Each core computes
q/k/v projections for its 128 channels, per-head causal attention for both
batches, and a partial output projection (its 128-channel slice of Wo); the
host sums the 8 partial projections.

Device-side design notes:
- All matmul operands are fp16 (PSUM accumulation stays f32): fp16 matmuls run
  at 1 cycle/row for any output width, including the 128-wide diagonal tiles
  and the PE transposes.
- LayerNorm mean-subtraction is folded into the weights on the host (a linear
  map), so on device only rstd = 1/sqrt(mean(q'^2)+eps) is needed
  (ACT Sqrt + DVE reciprocal).
- Scores are computed as s[k, q] (k tokens on partitions); exp runs on ACT in
  batched [128, 2, 512] tiles (paired k-tiles) to amortize instruction
  overhead. Fully-masked k-tiles are skipped by loop bounds; diagonal tiles
  zero the k>q triangle of exp(s) via affine_select on Pool.
- The attention output is accumulated as ao[q, dh] (q on partitions):
  matmul(lhsT=exp(s) tile, rhs=v_aug). A ones-column appended to v gives the
  softmax denominator per q *partition*, so normalization is a single cheap
  tensor_scalar per q-subtile with a per-partition reciprocal. ao is then
  transposed back to [ch, q] on the PE for the output projection.
- Emission interleaves batch-0 attention with batch-1 projection (and defers
  batch-0's attention@v until after batch-1's projection) so the ACT-bound
  softmax work overlaps the PE-bound projection work. Output-projection
  matmul+DMA pairs are deferred and drip-fed between later units so the
  in-order PE queue never stalls on a PSUM slot waiting for a DMA.
"""

import numpy as np

import concourse.bass as bass
import concourse.mybir as mybir
import concourse.tile as tile
from concourse.bass_utils import run_bass_kernel_spmd
from concourse.masks import make_identity

F32 = mybir.dt.float32
F16 = mybir.dt.float16

B, S, D, H = 2, 2048, 1024, 16
DH = D // H          # 64
NCORES = 8
HPC = H // NCORES    # 2 heads per core
CH = HPC * DH        # 128 channels per core
T = B * S            # 4096 tokens
DCH = D // 128       # 8 contraction chunks
TT = T // 128        # 32 token tiles
TTB = TT // B        # 16 token tiles per batch
QW = 512             # q-chunk width
QC = S // QW         # 4 q-chunks per batch
KTB = S // 128       # 16 k-tiles per batch
EPS = 1e-5

AF = mybir.ActivationFunctionType
ALU = mybir.AluOpType
GUARD = 2


def _split_drain_waits(nc):
    """walrus in this env only accepts one sync-wait per instruction;
    hoist extra waits onto preceding single-wait NOPs on the same engine."""
    for f in nc.m.functions:
        for blk in f.blocks:
            new_insts = []
            for inst in blk.instructions:
                si = getattr(inst, "sync_info", None)
                if si is not None and si.on_wait and len(si.on_wait) > 1:
                    waits = list(si.on_wait)
                    for j, w in enumerate(waits[:-1]):
                        new_insts.append(
                            mybir.InstNoOp(
                                name=f"{inst.name}-dwsplit{j}",
                                engine=inst.engine,
                                ins=[],
                                outs=[],
                                sync_info=mybir.SyncInfo(on_wait=[w], on_update=[]),
                            )
                        )
                    si.on_wait = [waits[-1]]
                    inst.sync_info = si
                new_insts.append(inst)
            blk.instructions[:] = new_insts


def _build(use_bias=False, debug_taps=False):
    nc = bass.Bass("TRN2", target_bir_lowering=False, debug=False)

    xt_d = nc.dram_tensor("xt", [D, T], F16, kind="ExternalInput")
    wqkvt_d = nc.dram_tensor("wqkvt", [D, 3 * CH], F16, kind="ExternalInput")
    bqkv_d = (
        nc.dram_tensor("bqkv", [1, 3 * CH], F32, kind="ExternalInput")
        if use_bias
        else None
    )
    wot_d = nc.dram_tensor("wot", [CH, D], F16, kind="ExternalInput")
    pot_d = nc.dram_tensor("pot", [D, T], F16, kind="ExternalOutput")
    if debug_taps:
        qT_dbg = nc.dram_tensor("qT_dbg", [128, T], F16, kind="ExternalOutput")
        kT_dbg = nc.dram_tensor("kT_dbg", [128, T], F16, kind="ExternalOutput")
        vaug_dbg = nc.dram_tensor("vaug_dbg", [128, TT * HPC * (DH + 1)], F16, kind="ExternalOutput")
        ex_dbg = nc.dram_tensor("ex_dbg", [128, QW], F16, kind="ExternalOutput")
        ao_dbg = nc.dram_tensor("ao_dbg", [128, 4 * HPC * DH], F16, kind="ExternalOutput")
        aot_dbg = nc.dram_tensor("aot_dbg", [128, QW], F16, kind="ExternalOutput")

    with tile.TileContext(nc) as tc:
        with (
            tc.tile_pool(name="const", bufs=1) as const_pool,
            tc.tile_pool(name="big", bufs=1) as big,
            tc.tile_pool(name="xt", bufs=2) as xpool,
            tc.tile_pool(name="qk", bufs=3) as qkpool,
            tc.tile_pool(name="ln", bufs=3) as lnpool,
            tc.tile_pool(name="qln", bufs=3) as qlnpool,
            tc.tile_pool(name="ex", bufs=48) as ex_pool,
            tc.tile_pool(name="ao", bufs=2) as ao_pool,
            tc.tile_pool(name="aot", bufs=3) as aot_pool,
            tc.tile_pool(name="posb", bufs=3) as posb_pool,
            tc.tile_pool(name="nrm", bufs=3) as nrm_pool,
            # PSUM budget (8 banks):
            #  ps_a: proj tiles (2x1536B) + transpose tiles (2x512B) = 2 banks
            #  ps_s: score tiles 4 x 1 bank = 4 banks (deep PE run-ahead)
            #  ps_o: attention accumulators / out-proj tiles 2 x 1 bank
            # The projection gets a dedicated pool: its slot releases must
            # never depend on exp (ACT) or the in-order ACT queue deadlocks
            # against qk-copies.
            tc.tile_pool(name="ps_a", bufs=2, space="PSUM") as ps_a_pool,
            tc.tile_pool(name="ps_s", bufs=3, space="PSUM") as ps_s_pool,
            tc.tile_pool(name="ps_t", bufs=1, space="PSUM") as ps_t_pool,
            tc.tile_pool(name="ps_o", bufs=2, space="PSUM") as po_pool,
        ):
            identity = const_pool.tile([128, 128], F32)
            make_identity(nc, identity)
            ident16 = const_pool.tile([128, 128], F16)
            nc.vector.tensor_copy(out=ident16, in_=identity)

            wqkv_sb = const_pool.tile([128, DCH, 3 * CH], F16)
            nc.sync.dma_start(
                out=wqkv_sb,
                in_=wqkvt_d.rearrange("(a p) c -> p a c", p=128),
            )
            if use_bias:
                bias_sb = const_pool.tile([128, 3 * CH], F32)
                nc.sync.dma_start(
                    out=bias_sb, in_=bqkv_d[0:1, :].to_broadcast([128, 3 * CH])
                )
            wo_sb = const_pool.tile([128, D], F16)
            nc.sync.dma_start(out=wo_sb, in_=wot_d[:, :])

            eps_sb = const_pool.tile([128, 1], F32)
            nc.vector.memset(eps_sb, EPS)
            if debug_taps:
                dbg_ao_sb = const_pool.tile([128, 512], F16)
                dbg_aot_sb = const_pool.tile([128, 512], F16)

            qT = big.tile([128, T], F16)
            kT = big.tile([128, T], F16)
            vaug = big.tile([128, TT, HPC, DH + 1], F16)
            nc.vector.memset(vaug[:, :, :, DH], 1.0)

            # -------- phase-1 unit: one 128-token tile of proj + LN + T --------
            pend_transp = []

            def p1_tile(t):
                u, half = t // 2, t % 2
                if half == 0:
                    xt_sb = xpool.tile([128, DCH, 256], F16, tag="xt")
                    p1_tile.xt_sb = xt_sb
                    nc.sync.dma_start(
                        out=xt_sb,
                        in_=xt_d[:, 256 * u : 256 * (u + 1)].rearrange(
                            "(a p) t -> p a t", p=128
                        ),
                    )
                xt_sb = p1_tile.xt_sb
                ps = ps_a_pool.tile([128, 512], F32, tag="a")
                for d in range(DCH):
                    nc.tensor.matmul(
                        ps[:, 0 : 3 * CH],
                        lhsT=xt_sb[:, d, 128 * half : 128 * (half + 1)],
                        rhs=wqkv_sb[:, d, :],
                        start=(d == 0),
                        stop=(d == DCH - 1),
                    )
                if use_bias:
                    nc.vector.tensor_add(out=ps, in0=ps, in1=bias_sb)
                # PSUM egress must be DVE/ACT (GPSIMD cannot access PSUM);
                # SBUF-only elementwise work goes to the otherwise-idle Pool.
                # Early tiles use ACT for the qk copy (no exp backlog yet).
                qk = qkpool.tile([128, 2 * CH], F16, tag="qk")
                if t < 10:
                    nc.scalar.copy(out=qk, in_=ps[:, 0 : 2 * CH])
                else:
                    nc.vector.tensor_copy(out=qk, in_=ps[:, 0 : 2 * CH])
                nc.vector.tensor_copy(
                    out=vaug[:, t, :, 0:DH],
                    in_=ps[:, 2 * CH : 3 * CH].rearrange("p (h x) -> p h x", x=DH),
                )
                # LN stats: sum of squares per 64-group (SBUF-only -> Pool)
                sq = lnpool.tile([128, 2 * CH], F16, tag="sq")
                nc.gpsimd.tensor_mul(out=sq, in0=qk, in1=qk)
                ssum = lnpool.tile([128, 4], F32, tag="ssum")
                nc.vector.reduce_sum(
                    out=ssum,
                    in_=sq.rearrange("p (g x) -> p g x", x=DH),
                    axis=mybir.AxisListType.X,
                )
                sd = lnpool.tile([128, 4], F32, tag="sd")
                nc.scalar.activation(
                    out=sd, in_=ssum, func=AF.Sqrt, scale=1.0 / DH, bias=eps_sb[:, :]
                )
                rstd = nrm_pool.tile([128, 4], F32, tag="rstd")
                nc.vector.reciprocal(out=rstd, in_=sd)
                rstd16 = nrm_pool.tile([128, 4], F16, tag="rstd16")
                nc.vector.tensor_copy(out=rstd16, in_=rstd)
                qln = qlnpool.tile([128, 2 * CH], F16, tag="qln")
                rstd_ap = rstd16[:, :]
                rstd_b = bass.AP(
                    tensor=rstd_ap.tensor,
                    offset=rstd_ap.offset,
                    ap=rstd_ap.ap + [[0, DH]],
                )
                nc.vector.tensor_mul(
                    out=qln.rearrange("p (g x) -> p g x", x=DH),
                    in0=qk.rearrange("p (g x) -> p g x", x=DH),
                    in1=rstd_b,
                )

                def transp():
                    pst = ps_t_pool.tile([128, 4, 128], F16, tag="t", name="pst")
                    for which, dst in ((0, qT), (1, kT)):
                        nc.tensor.transpose(
                            pst[:, which, :],
                            qln[:, CH * which : CH * (which + 1)],
                            ident16,
                        )
                        nc.vector.tensor_copy(
                            out=dst[:, 128 * t : 128 * (t + 1)], in_=pst[:, which, :]
                        )

                # defer the transposes ~2 tiles so the LN chain latency is
                # hidden instead of stalling the in-order PE queue
                pend_transp.append((t, transp))
                if len(pend_transp) > 2:
                    pend_transp.pop(0)[1]()

            # -------- phase-2 scores+exp for one k-tile of (b, qc, h) --------
            ex_map = {}

            def p2_group(b, qc, h, kt):
                q0 = b * S + qc * QW
                ps_s = ps_s_pool.tile([128, QW], F32, tag="s")
                ex = ex_pool.tile([128, QW], F16, tag="ex")
                c0 = max(0, (kt - 4 * qc) * 128)
                nc.tensor.matmul(
                    ps_s[:, c0:QW],
                    lhsT=kT[
                        DH * h : DH * (h + 1),
                        b * S + 128 * kt : b * S + 128 * (kt + 1),
                    ],
                    rhs=qT[DH * h : DH * (h + 1), q0 + c0 : q0 + QW],
                    start=True,
                    stop=True,
                )
                ex_map[(b, qc, h, kt)] = ex
                if debug_taps and (b, qc, h, kt) == (0, 1, 0, 2):
                    p2_group.ex_tap = ex
                nc.scalar.activation(
                    out=ex[:, c0:QW],
                    in_=ps_s[:, c0:QW],
                    func=AF.Exp,
                    scale=1.0 / np.sqrt(DH),
                )
                j = kt - 4 * qc
                if j >= 0:
                    d0 = 128 * j
                    nc.gpsimd.affine_select(
                        out=ex[:, d0 : d0 + 128],
                        in_=ex[:, d0 : d0 + 128],
                        compare_op=ALU.is_ge,
                        fill=0.0,
                        base=0,
                        pattern=[[1, 128]],
                        channel_multiplier=-1,
                    )
                # ACT-ns estimate for the pacing counter
                return (QW - c0) * 0.833 + 185.0

            # -------- phase-2 attention@v chunks + normalize ----------------
            po_map = {}

            def p2_attn_chunk(b, qc, h, j, ao_sb):
                # one q-subtile per chunk: its accumulation group is emitted
                # contiguously (interleaved PSUM groups mis-accumulate on HW)
                if j == 0:
                    po_map[(b, qc, h)] = po_pool.tile([128, 4, 128], F32, tag="po", name="po")
                po = po_map[(b, qc, h)]
                for kt in range(0, 4 * qc + j + 1):
                    ex = ex_map[(b, qc, h, kt)]
                    if j == 3:
                        ex_map.pop((b, qc, h, kt))
                    nc.tensor.matmul(
                        po[:, j, 0 : DH + 1],
                        lhsT=ex[:, 128 * j : 128 * (j + 1)],
                        rhs=vaug[:, b * KTB + kt, h, :],
                        start=(kt == 0),
                        stop=(kt == 4 * qc + j),
                    )
                if j == 3:
                    # single PSUM egress (DVE), then normalize on Pool in SBUF
                    posb = lnpool.tile([128, 4, DH + 1], F32, tag="posb")
                    nc.vector.tensor_copy(out=posb, in_=po[:, :, 0 : DH + 1])
                    rd = nrm_pool.tile([128, 4], F32, tag="rd")
                    nc.vector.reciprocal(out=rd, in_=posb[:, :, DH])
                    for j in range(4):
                        nc.gpsimd.tensor_scalar(
                            out=ao_sb[:, j, h, :],
                            in0=posb[:, j, 0:DH],
                            scalar1=rd[:, j : j + 1],
                            scalar2=None,
                            op0=ALU.mult,
                        )
                    po_map.pop((b, qc, h))

            # -------- phase-2 epilogue: ao transpose + deferred out-proj ----
            def p2_transpose(b, qc, ao_sb, aot_box):
                if debug_taps and (b, qc) == (0, 1):
                    nc.vector.tensor_copy(
                        out=dbg_ao_sb, in_=ao_sb.rearrange("p a b c -> p (a b c)")
                    )
                aoT = aot_pool.tile([128, QW], F16, tag="aot")
                ps_ao = ps_t_pool.tile([128, 4, 128], F16, tag="t", name="ps_ao")
                for j in range(4):
                    nc.tensor.transpose(
                        ps_ao[:, j, :],
                        ao_sb[:, j, :, :].rearrange("p h x -> p (h x)"),
                        ident16,
                    )
                nc.vector.tensor_copy(
                    out=aoT, in_=ps_ao.rearrange("p a x -> p (a x)")
                )
                if debug_taps and (b, qc) == (0, 1):
                    nc.vector.tensor_copy(out=dbg_aot_sb, in_=aoT)
                aot_box.append(aoT)
                aot_box.append(posb_pool.tile([128, DCH, QW], F16, tag="posb", name="po_sb"))

            def p2_outproj(b, qc, dc, aot_box, eng):
                q0 = b * S + qc * QW
                aoT, po_sb = aot_box
                # short-lived: shares the transpose pool rotation, NOT the
                # long-lived attention-accumulator pool (slot reuse there
                # races with later-emitted accumulation chunks)
                ps_po = ps_t_pool.tile([128, QW], F32, tag="t", name="ps_po")
                nc.tensor.matmul(
                    ps_po,
                    lhsT=wo_sb[:, 128 * dc : 128 * (dc + 1)],
                    rhs=aoT,
                    start=True,
                    stop=True,
                )
                if eng is nc.scalar:
                    eng.copy(out=po_sb[:, dc, :], in_=ps_po)
                else:
                    eng.tensor_copy(out=po_sb[:, dc, :], in_=ps_po)
                if dc == DCH - 1:
                    nc.sync.dma_start(
                        out=pot_d[:, q0 : q0 + QW].rearrange("(a p) t -> p a t", p=128),
                        in_=po_sb,
                    )

            # ================= interleaved emission =================
            # A continuous stream of scores+exp groups keeps ACT (the softmax
            # engine) saturated; attention chunks / epilogues are dripped a
            # couple of groups behind their exps, and projection tiles are
            # emitted lazily right before the first unit that needs them.
            state = {"group": 0}
            filler = []  # list of dicts: {fn, guard}
            emitted_p1 = set()

            def push(fn, guard=0):
                filler.append({"fn": fn, "guard": guard})

            def drip():
                while filler and filler[0]["guard"] <= state["group"]:
                    filler.pop(0)["fn"]()

            def ensure_p1(tiles):
                for t in tiles:
                    if t not in emitted_p1:
                        emitted_p1.add(t)
                        p1_tile(t)
                # flush deferred transposes for any required tile: scores
                # reading qT/kT must be emitted AFTER the transpose writes
                # (program order defines the dataflow)
                tset = set(tiles)
                keep = []
                for tt, fn in pend_transp:
                    if tt in tset:
                        fn()
                    else:
                        keep.append((tt, fn))
                pend_transp[:] = keep

            units = []
            for qc in range(QC):
                units += [(0, qc), (1, qc)]
            # p1 tiles to interleave during each unit: the tiles the NEXT
            # unit needs (prelude covers the first unit's)
            p1_per_unit = [
                list(range(u_next[0] * TTB + 4 * u_next[1], u_next[0] * TTB + 4 * u_next[1] + 4))
                for u_next in units[1:]
            ] + [[]]

            debug_ao = []
            for ui, (b, qc) in enumerate(units):
                ensure_p1([b * TTB + t for t in range(4 * qc + 4)])
                ao_sb = ao_pool.tile([128, 4, HPC, DH], F16, tag="ao")
                if debug_taps and (b, qc) == (0, 1) and not debug_ao:
                    debug_ao.append(ao_sb)
                n_kt = 4 * (qc + 1)
                n_groups = HPC * n_kt
                p1_list = [t for t in p1_per_unit[ui] if t not in emitted_p1]
                stride = max(1, n_groups // max(1, len(p1_list)))
                g_in_unit = 0
                for h in range(HPC):
                    for kt in range(n_kt):
                        p2_group(b, qc, h, kt)
                        state["group"] += 1
                        g_in_unit += 1
                        if p1_list and g_in_unit % stride == 0:
                            tt = p1_list.pop(0)
                            emitted_p1.add(tt)
                            p1_tile(tt)
                        drip()
                    # attention chunks (one q-subtile each), right behind exps
                    for j in range(4):
                        push(
                            (
                                lambda a, c, e, jj, ao: lambda: p2_attn_chunk(
                                    a, c, e, jj, ao
                                )
                            )(b, qc, h, j, ao_sb),
                            guard=state["group"] + GUARD,
                        )
                ensure_p1(p1_list)
                # epilogue: transpose then 8 out-proj drips
                aot_box = []
                push(
                    (lambda a, c, ao, box: lambda: p2_transpose(a, c, ao, box))(
                        b, qc, ao_sb, aot_box
                    )
                )
                engs = (nc.vector, nc.vector)
                for dc in range(DCH):
                    push(
                        (
                            lambda a, c, d, box, eng: lambda: p2_outproj(
                                a, c, d, box, eng
                            )
                        )(b, qc, dc, aot_box, engs[dc % 2]),
                    )
            # tail: drain whatever is left, guards no longer apply
            while filler:
                filler.pop(0)["fn"]()
            if debug_taps:
                nc.sync.dma_start(out=qT_dbg[:, :], in_=qT)
                nc.sync.dma_start(out=kT_dbg[:, :], in_=kT)
                nc.sync.dma_start(out=vaug_dbg[:, :], in_=vaug.rearrange("p a b c -> p (a b c)"))
                nc.sync.dma_start(out=ex_dbg[:, :], in_=p2_group.ex_tap)
                nc.sync.dma_start(out=ao_dbg[:, :], in_=dbg_ao_sb)
                nc.sync.dma_start(out=aot_dbg[:, :], in_=dbg_aot_sb)

    _split_drain_waits(nc)
    return nc


_NC_CACHE = {}


def _get_nc(use_bias=False):
    if use_bias not in _NC_CACHE:
        _NC_CACHE[use_bias] = _build(use_bias)
    return _NC_CACHE[use_bias]


def _prep_inputs(x, Wq, bq, Wk, bk, Wv, bv, Wo):
    xt = np.ascontiguousarray(x.reshape(T, D).T).astype(np.float16)
    in_maps = []
    for c in range(NCORES):
        sl = slice(CH * c, CH * (c + 1))
        wq_c = np.array(Wq[sl, :], dtype=np.float32)
        bq_c = np.array(bq[sl], dtype=np.float32)
        wk_c = np.array(Wk[sl, :], dtype=np.float32)
        bk_c = np.array(bk[sl], dtype=np.float32)
        # fold the LayerNorm mean-subtraction (a linear map) into W and b
        for h in range(HPC):
            blk = slice(DH * h, DH * (h + 1))
            wq_c[blk, :] -= wq_c[blk, :].mean(axis=0, keepdims=True)
            bq_c[blk] -= bq_c[blk].mean()
            wk_c[blk, :] -= wk_c[blk, :].mean(axis=0, keepdims=True)
            bk_c[blk] -= bk_c[blk].mean()
        wv_c = np.array(Wv[sl, :], dtype=np.float32)
        bv_c = np.array(bv[sl], dtype=np.float32)
        wqkvt = np.ascontiguousarray(
            np.concatenate([wq_c, wk_c, wv_c], axis=0).T
        ).astype(np.float16)
        bqkv = np.concatenate([bq_c, bk_c, bv_c])[None, :].astype(np.float32)
        wot = np.ascontiguousarray(Wo[:, sl].T).astype(np.float16)
        in_maps.append({"xt": xt, "wqkvt": wqkvt, "bqkv": bqkv, "wot": wot})
    return in_maps


def kernel(x, mask, Wq, bq, Wk, bk, Wv, bv, Wo, bo, _trace=False):
    x = np.asarray(x, dtype=np.float32)
    in_maps = _prep_inputs(
        x,
        np.asarray(Wq),
        np.asarray(bq),
        np.asarray(Wk),
        np.asarray(bk),
        np.asarray(Wv),
        np.asarray(bv),
        np.asarray(Wo),
    )
    use_bias = bool(
        np.any(np.asarray(bq)) or np.any(np.asarray(bk)) or np.any(np.asarray(bv))
    )
    if not use_bias:
        for m in in_maps:
            del m["bqkv"]
    nc = _get_nc(use_bias)
    res = run_bass_kernel_spmd(
        nc, in_maps, core_ids=list(range(NCORES)), trace=_trace
    )
    pot = np.zeros((D, T), np.float32)
    for c in range(NCORES):
        pot += res.results[c]["pot"].astype(np.float32)
    out = pot.T + np.asarray(bo, dtype=np.float32)[None, :]
    out = out.reshape(B, S, D)
    if _trace:
        return out, res
    return out
